# revision 1
# baseline (speedup 1.0000x reference)
"""PhysicsAttention (structured mesh 2D) Trainium2 kernel.

Data-parallel over batch: each of the 8 NeuronCores processes one batch
element end-to-end (no collectives).

Per-core pipeline (one batch element, mesh 128x128, N=16384 pixels):
  phase A (per row-tile of 4 image rows = 512 px):
    conv_x   : 3x3/128->512 conv as 9 shifted f32r matmuls, channel-major out
    logits   : block-diag slice_w matmul (2 heads per 128-row block)
    e=exp()  : ACT with per-partition 1/temp scale + folded bias
    eT       : PE transposes to pixel-major, per 128-px chunk
    softmax  : row sums (DVE reduce) + reciprocal -> w = e * (1/s)
    conv_fx  : same conv, pixel-major out (x-window slices as stationary)
    slice_tok: accumulate st[g,c] += wT.T @ fx  and norm[g] += wT.T @ 1
    w_chm    : PE transpose w back to channel-major for phase C
  phase B (tiny): normalize slice tokens, q/k/v, 64-token attention, fold
    out_slice with out_w into M[g, d]
  phase C: outT[d, n] = sum_g M[g,:].T @ w_chm[g, n]  (K=512 over 4 blocks)

Host side: pads/transposes x, reorders weights, folds conv_x bias +
slice bias + temperature into the exp() scale/bias vectors, adds out_b.
"""

import numpy as np
import ml_dtypes
from contextlib import ExitStack

B = 8
HM = WM = 128
DIM = 128
HEADS = 8
DH = 64
G = 64
INNER = 512
N = HM * WM
NCORES = 8
RT = 32  # row tiles (4 image rows each)

_CACHE = {}


def _build():
    import concourse.bass as bass
    import concourse.tile as tile
    from concourse import bacc, mybir
    from concourse.masks import make_identity

    f32 = mybir.dt.float32
    f32r = mybir.dt.float32r
    bf16 = mybir.dt.bfloat16
    AF = mybir.ActivationFunctionType
    AX = mybir.AxisListType

    nc = bacc.Bacc("TRN2", target_bir_lowering=False, debug=False)
    xTp = nc.dram_tensor("xTp", [128, 130, 130], f32r, kind="ExternalInput").ap()
    wx = nc.dram_tensor("wx", [128, 9 * 512], f32r, kind="ExternalInput").ap()
    wfx = nc.dram_tensor("wfx", [128, 9 * 512], f32r, kind="ExternalInput").ap()
    swbd = nc.dram_tensor("swbd", [128, 128], f32r, kind="ExternalInput").ap()
    actv = nc.dram_tensor("actv", [128, 8], f32, kind="ExternalInput").ap()
    wqkv = nc.dram_tensor("wqkv", [128, 192], f32, kind="ExternalInput").ap()
    owt = nc.dram_tensor("owt", [64, 1024], f32, kind="ExternalInput").ap()
    bfxp = nc.dram_tensor("bfxp", [1, 512], f32, kind="ExternalInput").ap()
    outT = nc.dram_tensor("outT", [128, 16384], f32, kind="ExternalOutput").ap()

    with tile.TileContext(nc) as tc, ExitStack() as top:
        consts = top.enter_context(tc.tile_pool(name="consts", bufs=1))
        wchmP = top.enter_context(tc.tile_pool(name="wchmP", bufs=1))

        wx_sb = consts.tile([128, 9 * 512], f32r)
        nc.sync.dma_start(wx_sb[:], wx[:])
        wfx_sb = consts.tile([128, 9 * 512], f32r)
        nc.sync.dma_start(wfx_sb[:], wfx[:])
        swbd_sb = consts.tile([128, 128], f32r)
        nc.sync.dma_start(swbd_sb[:], swbd[:])
        actv_sb = consts.tile([128, 8], f32)
        nc.sync.dma_start(actv_sb[:], actv[:])
        wqkv_sb = consts.tile([128, 192], f32)
        nc.sync.dma_start(wqkv_sb[:], wqkv[:])
        owt_sb = consts.tile([64, 1024], f32)
        nc.sync.dma_start(owt_sb[:], owt[:])
        bfx_sb = consts.tile([1, 512], f32)
        nc.sync.dma_start(bfx_sb[:], bfxp[:])
        idbf = consts.tile([128, 128], bf16)
        make_identity(nc, idbf[:])
        idf32 = consts.tile([128, 128], f32)
        make_identity(nc, idf32[:])
        M_sb = consts.tile([128, 512], bf16)

        Wchm = wchmP.tile([128, 4 * 16384], bf16)

        with tc.tile_pool(name="stP", bufs=1, space="PSUM") as stP:
            # two banks, each holding two (g,c)-pair regions of width 129:
            # cols [0:128) = slice_token pair, col 128 = norm (ones-column)
            psum_st0 = stP.tile([128, 258], f32, tag="st0")
            psum_st1 = stP.tile([128, 258], f32, tag="st1")
            st_banks = (psum_st0, psum_st1)

            with tc.tile_pool(name="xwin", bufs=2) as xwinP, \
                 tc.tile_pool(name="sbA", bufs=3) as sbA, \
                 tc.tile_pool(name="psA", bufs=3, space="PSUM") as psA:
                for t in range(RT):
                    w6 = xwinP.tile([128, 6, 130], f32r)
                    nc.sync.dma_start(w6[:], xTp[:, 4 * t: 4 * t + 6, :])
                    e_list = []
                    for q in range(4):
                        pxm = psA.tile([128, 512], f32, tag="pA")
                        for tap in range(9):
                            ky, kx = tap // 3, tap % 3
                            nc.tensor.matmul(
                                pxm[:],
                                wx_sb[:, tap * 512 + q * 128: tap * 512 + (q + 1) * 128],
                                w6[:, ky: ky + 4, kx: kx + 128],
                                start=(tap == 0), stop=(tap == 8))
                        xm = sbA.tile([128, 512], f32r, tag="xm", bufs=2)
                        nc.vector.tensor_copy(xm[:], pxm[:])
                        plg = psA.tile([128, 512], f32, tag="pB")
                        nc.tensor.matmul(plg[:], swbd_sb[:], xm[:], start=True, stop=True)
                        e_q = sbA.tile([128, 512], bf16, tag="e", bufs=6)
                        nc.scalar.activation(e_q[:], plg[:], AF.Exp,
                                             bias=actv_sb[:, 4 + q: 5 + q],
                                             scale=actv_sb[:, q: q + 1])
                        e_list.append(e_q)
                    for k in range(4):
                        gch = 4 * t + k
                        peT = psA.tile([128, 512], bf16, tag="pB")
                        for q in range(4):
                            nc.tensor.transpose(peT[:, q * 128:(q + 1) * 128],
                                                e_list[q][:, k * 128:(k + 1) * 128],
                                                idbf[:])
                        s_k = sbA.tile([128, 8], f32, tag="s", bufs=4)
                        nc.vector.reduce_sum(
                            s_k[:], peT[:].rearrange("p (h g) -> p h g", h=8),
                            axis=AX.X)
                        r_k = sbA.tile([128, 8], f32, tag="r", bufs=4)
                        nc.vector.reciprocal(r_k[:], s_k[:])
                        wT = sbA.tile([128, 512], bf16, tag="wT", bufs=3)
                        r_b = bass.AP(tensor=r_k[:].tensor, offset=r_k[:].offset,
                                      ap=[r_k[:].ap[0], [1, 8], [0, 64]])
                        nc.vector.tensor_mul(wT[:], peT[:], r_b)
                        pfx = psA.tile([128, 512], f32, tag="pA")
                        for tap in range(9):
                            ky, kx = tap // 3, tap % 3
                            nc.tensor.matmul(
                                pfx[:],
                                w6[:, k + ky, kx: kx + 128],
                                wfx_sb[:, tap * 512:(tap + 1) * 512],
                                start=(tap == 0), stop=(tap == 8))
                        fx = sbA.tile([128, 4, 129], bf16, tag="fx", bufs=3)
                        nc.scalar.activation(
                            fx[:, :, 0:128],
                            pfx[:].rearrange("p (q n) -> p q n", q=4), AF.Copy)
                        nc.vector.memset(fx[:, :, 128:129], 1.0)
                        for p in range(4):
                            nc.tensor.matmul(
                                st_banks[p // 2][:, (p % 2) * 129:(p % 2) * 129 + 129],
                                wT[:, p * 128:(p + 1) * 128],
                                fx[:, p, :],
                                start=(gch == 0 and p % 2 == 0),
                                stop=(gch == 127 and p % 2 == 1))
                        pwc = psA.tile([128, 512], bf16, tag="pB")
                        for q in range(4):
                            nc.tensor.transpose(pwc[:, q * 128:(q + 1) * 128],
                                                wT[:, q * 128:(q + 1) * 128],
                                                idbf[:])
                        nc.vector.tensor_copy(
                            Wchm[:].rearrange("p (q n) -> p q n", q=4)[:, :, gch * 128:(gch + 1) * 128],
                            pwc[:])

            # ---- phase B ----
            with tc.tile_pool(name="sbB", bufs=2) as sbB, \
                 tc.tile_pool(name="psB", bufs=2, space="PSUM") as psB, \
                 tc.tile_pool(name="psM", bufs=1, space="PSUM") as psM:
                norm_c = sbB.tile([128, 4], f32, bufs=1)
                for b_ in range(2):
                    src = st_banks[b_][:]
                    nc.vector.tensor_copy(
                        norm_c[:, 2 * b_: 2 * b_ + 2],
                        bass.AP(tensor=src.tensor, offset=src.offset + 128,
                                ap=[src.ap[0], [129, 2]]))
                nflat = sbB.tile([1, 512], f32, bufs=1)
                for p in range(4):
                    pnT = psB.tile([1, 128], f32, tag="pnT", bufs=1)
                    nc.tensor.transpose(pnT[:], norm_c[:, p: p + 1], idf32[:])
                    nc.vector.tensor_copy(nflat[0:1, p * 128:(p + 1) * 128], pnT[:])
                pbfx = psB.tile([128, 512], f32, tag="pbfx", bufs=1)
                for p in range(4):
                    nc.tensor.matmul(pbfx[:, p * 128:(p + 1) * 128],
                                     nflat[0:1, p * 128:(p + 1) * 128],
                                     bfx_sb[0:1, p * 128:(p + 1) * 128],
                                     start=(p == 0), stop=(p == 3))
                bfxo = sbB.tile([128, 512], f32, bufs=1)
                nc.vector.tensor_copy(bfxo[:], pbfx[:])
                ne = sbB.tile([128, 4], f32, bufs=1)
                nc.vector.tensor_scalar_add(ne[:], norm_c[:], 1e-5)
                rn = sbB.tile([128, 4], f32, bufs=1)
                nc.vector.reciprocal(rn[:], ne[:])
                stn = sbB.tile([128, 512], f32, bufs=1)
                for p in range(4):
                    nc.vector.tensor_add(
                        stn[:, p * 128:(p + 1) * 128],
                        st_banks[p // 2][:, (p % 2) * 129:(p % 2) * 129 + 128],
                        bfxo[:, p * 128:(p + 1) * 128])
                    nc.vector.tensor_scalar_mul(
                        stn[:, p * 128:(p + 1) * 128],
                        stn[:, p * 128:(p + 1) * 128],
                        rn[:, p: p + 1])
                pstnT = psB.tile([128, 512], f32, tag="pstnT", bufs=1)
                for p in range(4):
                    nc.tensor.transpose(pstnT[:, p * 128:(p + 1) * 128],
                                        stn[:, p * 128:(p + 1) * 128], idf32[:])
                stnT = sbB.tile([128, 512], f32, bufs=1)
                nc.vector.tensor_copy(stnT[:], pstnT[:])
                qkv_list = []
                for h in range(8):
                    p_, s_ = h // 2, h % 2
                    st_h = stnT[s_ * 64:(s_ + 1) * 64,
                                p_ * 128 + s_ * 64: p_ * 128 + s_ * 64 + 64]
                    wq_h = wqkv_sb[s_ * 64:(s_ + 1) * 64, :]
                    pq = psB.tile([64, 192], f32, tag="pqkv", bufs=1)
                    nc.tensor.matmul(pq[:, 0:64], wq_h[:, 0:64], st_h, start=True, stop=True)
                    nc.tensor.matmul(pq[:, 64:128], wq_h[:, 64:128], st_h, start=True, stop=True)
                    nc.tensor.matmul(pq[:, 128:192], st_h, wq_h[:, 128:192], start=True, stop=True)
                    qkv = sbB.tile([64, 192], f32, tag="qkv", bufs=8)
                    nc.vector.tensor_copy(qkv[:], pq[:])
                    qkv_list.append(qkv)
                pM = psM.tile([128, 512], f32)
                for h in range(8):
                    p_, s_ = h // 2, h % 2
                    qkv = qkv_list[h]
                    pa = psB.tile([64, 64], f32, tag="pa", bufs=1)
                    nc.tensor.matmul(pa[:], qkv[:, 0:64], qkv[:, 64:128], start=True, stop=True)
                    mx = sbB.tile([64, 1], f32, tag="mx")
                    nc.vector.reduce_max(mx[:], pa[:], axis=AX.X)
                    mxs = sbB.tile([64, 1], f32, tag="mxs")
                    nc.vector.tensor_scalar_mul(mxs[:], mx[:], -0.125)
                    aw = sbB.tile([64, 64], f32, tag="aw")
                    sa = sbB.tile([64, 1], f32, tag="sa")
                    nc.scalar.activation(aw[:], pa[:], AF.Exp,
                                         bias=mxs[:], scale=0.125, accum_out=sa[:])
                    rsa = sbB.tile([64, 1], f32, tag="rsa")
                    nc.vector.reciprocal(rsa[:], sa[:])
                    awn = sbB.tile([64, 64], f32, tag="awn")
                    nc.vector.tensor_scalar_mul(awn[:], aw[:], rsa[:])
                    paT = psB.tile([64, 64], f32, tag="pa", bufs=1)
                    nc.tensor.transpose(paT[:], awn[:], idf32[0:64, 0:64])
                    awT = sbB.tile([64, 64], f32, tag="awT")
                    nc.vector.tensor_copy(awT[:], paT[:])
                    poT = psB.tile([64, 64], f32, tag="pa", bufs=1)
                    nc.tensor.matmul(poT[:], qkv[:, 128:192], awT[:], start=True, stop=True)
                    oT = sbB.tile([64, 64], f32, tag="oT")
                    nc.vector.tensor_copy(oT[:], poT[:])
                    nc.tensor.matmul(pM[s_ * 64:(s_ + 1) * 64, p_ * 128:(p_ + 1) * 128],
                                     oT[:], owt_sb[:, h * 128:(h + 1) * 128],
                                     start=True, stop=True)
                nc.vector.tensor_copy(M_sb[:], pM[:])

        # ---- phase C ----
        with tc.tile_pool(name="sbC", bufs=3) as sbC, \
             tc.tile_pool(name="psC", bufs=3, space="PSUM") as psC:
            for i in range(32):
                po = psC.tile([128, 512], f32)
                for p in range(4):
                    nc.tensor.matmul(
                        po[:], M_sb[:, p * 128:(p + 1) * 128],
                        Wchm[:, p * 16384 + i * 512: p * 16384 + (i + 1) * 512],
                        start=(p == 0), stop=(p == 3))
                ob = sbC.tile([128, 512], f32)
                nc.vector.tensor_copy(ob[:], po[:])
                nc.sync.dma_start(outT[:, i * 512:(i + 1) * 512], ob[:])

    nc.compile()
    return nc


def _prep(inputs):
    x = np.asarray(inputs["x"], dtype=np.float32)
    conv_fx_w = np.asarray(inputs["conv_fx_w"], dtype=np.float32)
    conv_fx_b = np.asarray(inputs["conv_fx_b"], dtype=np.float32)
    conv_x_w = np.asarray(inputs["conv_x_w"], dtype=np.float32)
    conv_x_b = np.asarray(inputs["conv_x_b"], dtype=np.float32)
    slice_w = np.asarray(inputs["slice_w"], dtype=np.float32)
    slice_b = np.asarray(inputs["slice_b"], dtype=np.float32)
    temperature = np.asarray(inputs["temperature"], dtype=np.float32)
    wq = np.asarray(inputs["wq"], dtype=np.float32)
    wk = np.asarray(inputs["wk"], dtype=np.float32)
    wv = np.asarray(inputs["wv"], dtype=np.float32)
    out_w = np.asarray(inputs["out_w"], dtype=np.float32)

    wx_np = np.ascontiguousarray(
        conv_x_w.transpose(2, 0, 1, 3).reshape(128, 9 * 512))
    wfx_np = np.ascontiguousarray(
        conv_fx_w.transpose(2, 0, 1, 3).reshape(128, 9 * 512))

    swbd_np = np.zeros((128, 128), np.float32)
    swbd_np[0:64, 0:64] = slice_w.T
    swbd_np[64:128, 64:128] = slice_w.T

    temp = np.clip(temperature.reshape(HEADS), 0.1, 5.0)
    actv_np = np.zeros((128, 8), np.float32)
    for q in range(4):
        for j in range(2):
            h = 2 * q + j
            bias_fold = slice_b + slice_w @ conv_x_b[h * 64:(h + 1) * 64]
            actv_np[j * 64:(j + 1) * 64, q] = 1.0 / temp[h]
            actv_np[j * 64:(j + 1) * 64, 4 + q] = bias_fold / temp[h]

    wqkv_half = np.concatenate([wq.T, wk.T, wv.T], axis=1).astype(np.float32)
    wqkv_np = np.vstack([wqkv_half, wqkv_half])
    owt_np = np.ascontiguousarray(
        out_w.T.reshape(8, 64, 128).transpose(1, 0, 2).reshape(64, 1024))
    bfx_np = np.ascontiguousarray(conv_fx_b.reshape(1, 512))

    in_maps = []
    for b in range(B):
        xi = x[b].reshape(HM, WM, DIM)
        xp = np.zeros((128, 130, 130), np.float32)
        xp[:, 1:129, 1:129] = xi.transpose(2, 0, 1)
        in_maps.append({
            "xTp": xp, "wx": wx_np, "wfx": wfx_np, "swbd": swbd_np,
            "actv": actv_np, "wqkv": wqkv_np, "owt": owt_np, "bfxp": bfx_np,
        })
    return in_maps


def kernel(**inputs):
    from concourse.bass_utils import run_bass_kernel_spmd

    if "nc" not in _CACHE:
        _CACHE["nc"] = _build()
    nc = _CACHE["nc"]

    in_maps = _prep(inputs)
    res = run_bass_kernel_spmd(nc, in_maps, core_ids=list(range(NCORES)))
    out_b = np.asarray(inputs["out_b"], dtype=np.float32)
    out = np.empty((B, N, DIM), np.float32)
    for b in range(B):
        out[b] = res.results[b]["outT"].T + out_b
    return out



# revision 3
# speedup vs baseline: 1.1706x; 1.1706x over previous
"""PhysicsAttention (structured mesh 2D) Trainium2 kernel.

Data-parallel over batch: each of the 8 NeuronCores processes one batch
element end-to-end (no collectives).

Per-core pipeline (one batch element, mesh 128x128, N=16384 pixels):
  phase A (per row-tile of 4 image rows = 512 px):
    logits   : 3x3/128->512 conv with slice_w folded into the conv
               weights (bf16), channel-major out -> exp() directly off
               PSUM with per-partition 1/temp scale + folded bias
    eT       : PE transposes to pixel-major, per 128-px chunk
    softmax  : row sums (DVE reduce) + reciprocal -> w = e * (1/s)
    conv_fx  : same conv (bf16), pixel-major out (x-window stationary)
    slice_tok: accumulate st[g,c] += wT.T @ fx  and norm[g] += wT.T @ 1
    w_chm    : PE transpose w back to channel-major for phase C
  phase B: normalize slice tokens, then head-batched q/k/v, 64-token
    attention (no max-sub: logits are ~1e-3), fold out_slice with out_w
    into M[g, d]
  phase C: outT[d, n] = sum_g M[g,:].T @ w_chm[g, n]  (K=512, 4 blocks)

Host side: pads/transposes x (bf16), folds slice_w into conv_x weights,
folds conv_x bias + slice bias + temperature into the exp() scale/bias
vectors, adds out_b.
"""

import numpy as np
import ml_dtypes
from contextlib import ExitStack

B = 8
HM = WM = 128
DIM = 128
HEADS = 8
DH = 64
G = 64
INNER = 512
N = HM * WM
NCORES = 8
RT = 32  # row tiles (4 image rows each)

_CACHE = {}


def _build():
    import concourse.bass as bass
    import concourse.tile as tile
    from concourse import bacc, mybir
    from concourse.masks import make_identity

    f32 = mybir.dt.float32
    f32r = mybir.dt.float32r
    bf16 = mybir.dt.bfloat16
    AF = mybir.ActivationFunctionType
    AX = mybir.AxisListType

    nc = bacc.Bacc("TRN2", target_bir_lowering=False, debug=False)
    xTp = nc.dram_tensor("xTp", [128, 130, 130], bf16, kind="ExternalInput").ap()
    wx = nc.dram_tensor("wx", [128, 9 * 512], bf16, kind="ExternalInput").ap()
    wfx = nc.dram_tensor("wfx", [128, 9 * 512], bf16, kind="ExternalInput").ap()
    actv = nc.dram_tensor("actv", [128, 8], f32, kind="ExternalInput").ap()
    wqkv = nc.dram_tensor("wqkv", [128, 192], f32, kind="ExternalInput").ap()
    owt = nc.dram_tensor("owt", [128, 1024], f32, kind="ExternalInput").ap()
    bfxp = nc.dram_tensor("bfxp", [1, 512], f32, kind="ExternalInput").ap()
    outT = nc.dram_tensor("outT", [128, 16384], f32, kind="ExternalOutput").ap()

    with tile.TileContext(nc) as tc, ExitStack() as top:
        consts = top.enter_context(tc.tile_pool(name="consts", bufs=1))
        wchmP = top.enter_context(tc.tile_pool(name="wchmP", bufs=1))

        # weight loads split into chunks so they spread over DMA queues
        wx_sb = consts.tile([128, 9 * 512], bf16)
        for c in range(4):
            nc.sync.dma_start(wx_sb[:, c * 1152:(c + 1) * 1152],
                              wx[:, c * 1152:(c + 1) * 1152])
        wfx_sb = consts.tile([128, 9 * 512], bf16)
        for c in range(4):
            nc.sync.dma_start(wfx_sb[:, c * 1152:(c + 1) * 1152],
                              wfx[:, c * 1152:(c + 1) * 1152])
        actv_sb = consts.tile([128, 8], f32)
        nc.sync.dma_start(actv_sb[:], actv[:])
        wqkv_sb = consts.tile([128, 192], f32)
        nc.sync.dma_start(wqkv_sb[:], wqkv[:])
        owt_sb = consts.tile([128, 1024], f32)
        nc.sync.dma_start(owt_sb[:], owt[:])
        bfx_sb = consts.tile([1, 512], f32)
        nc.sync.dma_start(bfx_sb[:], bfxp[:])
        idbf = consts.tile([128, 128], bf16)
        make_identity(nc, idbf[:])
        idf32 = consts.tile([128, 128], f32)
        make_identity(nc, idf32[:])
        ones_sb = consts.tile([128, 1], f32)
        nc.vector.memset(ones_sb[:], 1.0)
        M_sb = consts.tile([128, 512], bf16)
        stn = consts.tile([128, 512], f32)

        Wchm = wchmP.tile([128, 4 * 16384], bf16)

        with tc.tile_pool(name="stP", bufs=1, space="PSUM") as stP:
            # two banks, each holding two (g,c)-pair regions of width 129:
            # cols [0:128) = slice_token pair, col 128 = norm (ones-column)
            psum_st0 = stP.tile([128, 258], f32, tag="st0")
            psum_st1 = stP.tile([128, 258], f32, tag="st1")
            st_banks = (psum_st0, psum_st1)

            with tc.tile_pool(name="xwin", bufs=2) as xwinP, \
                 tc.tile_pool(name="sbA", bufs=3) as sbA, \
                 tc.tile_pool(name="psA", bufs=3, space="PSUM") as psA:
                for t in range(RT):
                    w6 = xwinP.tile([128, 6, 130], bf16)
                    nc.sync.dma_start(w6[:], xTp[:, 4 * t: 4 * t + 6, :])
                    e_list = []
                    for q in range(4):
                        pxm = psA.tile([128, 512], f32, tag="pA")
                        for tap in range(9):
                            ky, kx = tap // 3, tap % 3
                            nc.tensor.matmul(
                                pxm[:],
                                wx_sb[:, tap * 512 + q * 128: tap * 512 + (q + 1) * 128],
                                w6[:, ky: ky + 4, kx: kx + 128],
                                start=(tap == 0), stop=(tap == 8))
                        e_q = sbA.tile([128, 512], bf16, tag="e", bufs=6)
                        nc.scalar.activation(e_q[:], pxm[:], AF.Exp,
                                             bias=actv_sb[:, 4 + q: 5 + q],
                                             scale=actv_sb[:, q: q + 1])
                        e_list.append(e_q)
                    for k in range(4):
                        gch = 4 * t + k
                        peT = psA.tile([128, 512], bf16, tag="pB")
                        for q in range(4):
                            nc.tensor.transpose(peT[:, q * 128:(q + 1) * 128],
                                                e_list[q][:, k * 128:(k + 1) * 128],
                                                idbf[:])
                        s_k = sbA.tile([128, 8], f32, tag="s", bufs=4)
                        nc.vector.reduce_sum(
                            s_k[:], peT[:].rearrange("p (h g) -> p h g", h=8),
                            axis=AX.X)
                        r_k = sbA.tile([128, 8], f32, tag="r", bufs=4)
                        nc.vector.reciprocal(r_k[:], s_k[:])
                        wT = sbA.tile([128, 512], bf16, tag="wT", bufs=3)
                        r_b = bass.AP(tensor=r_k[:].tensor, offset=r_k[:].offset,
                                      ap=[r_k[:].ap[0], [1, 8], [0, 64]])
                        nc.vector.tensor_mul(wT[:], peT[:], r_b)
                        pfx = psA.tile([128, 512], f32, tag="pA")
                        for tap in range(9):
                            ky, kx = tap // 3, tap % 3
                            nc.tensor.matmul(
                                pfx[:],
                                w6[:, k + ky, kx: kx + 128],
                                wfx_sb[:, tap * 512:(tap + 1) * 512],
                                start=(tap == 0), stop=(tap == 8))
                        fx = sbA.tile([128, 4, 129], bf16, tag="fx", bufs=3)
                        nc.scalar.activation(
                            fx[:, :, 0:128],
                            pfx[:].rearrange("p (q n) -> p q n", q=4), AF.Copy)
                        nc.vector.memset(fx[:, :, 128:129], 1.0)
                        for p in range(4):
                            nc.tensor.matmul(
                                st_banks[p // 2][:, (p % 2) * 129:(p % 2) * 129 + 129],
                                wT[:, p * 128:(p + 1) * 128],
                                fx[:, p, :],
                                start=(gch == 0 and p % 2 == 0),
                                stop=(gch == 127 and p % 2 == 1))
                        pwc = psA.tile([128, 512], bf16, tag="pB")
                        for q in range(4):
                            nc.tensor.transpose(pwc[:, q * 128:(q + 1) * 128],
                                                wT[:, q * 128:(q + 1) * 128],
                                                idbf[:])
                        nc.vector.tensor_copy(
                            Wchm[:].rearrange("p (q n) -> p q n", q=4)[:, :, gch * 128:(gch + 1) * 128],
                            pwc[:])

            # ---- phase B part 1: normalized slice tokens (needs st PSUM) ----
            with tc.tile_pool(name="sbB1", bufs=1) as sbB1, \
                 tc.tile_pool(name="psB1", bufs=1, space="PSUM") as psB1:
                norm_c = sbB1.tile([128, 4], f32)
                for b_ in range(2):
                    src = st_banks[b_][:]
                    nc.vector.tensor_copy(
                        norm_c[:, 2 * b_: 2 * b_ + 2],
                        bass.AP(tensor=src.tensor, offset=src.offset + 128,
                                ap=[src.ap[0], [129, 2]]))
                nflat = sbB1.tile([1, 512], f32)
                for p in range(4):
                    pnT = psB1.tile([1, 128], f32, tag="pnT")
                    nc.tensor.transpose(pnT[:], norm_c[:, p: p + 1], idf32[:])
                    nc.vector.tensor_copy(nflat[0:1, p * 128:(p + 1) * 128], pnT[:])
                pbfx = psB1.tile([128, 512], f32, tag="pbfx")
                for p in range(4):
                    nc.tensor.matmul(pbfx[:, p * 128:(p + 1) * 128],
                                     nflat[0:1, p * 128:(p + 1) * 128],
                                     bfx_sb[0:1, p * 128:(p + 1) * 128],
                                     start=(p == 0), stop=(p == 3))
                bfxo = sbB1.tile([128, 512], f32)
                nc.vector.tensor_copy(bfxo[:], pbfx[:])
                ne = sbB1.tile([128, 4], f32)
                nc.vector.tensor_scalar_add(ne[:], norm_c[:], 1e-5)
                rn = sbB1.tile([128, 4], f32)
                nc.vector.reciprocal(rn[:], ne[:])
                for p in range(4):
                    nc.vector.tensor_add(
                        stn[:, p * 128:(p + 1) * 128],
                        st_banks[p // 2][:, (p % 2) * 129:(p % 2) * 129 + 128],
                        bfxo[:, p * 128:(p + 1) * 128])
                    nc.vector.tensor_scalar_mul(
                        stn[:, p * 128:(p + 1) * 128],
                        stn[:, p * 128:(p + 1) * 128],
                        rn[:, p: p + 1])

        # ---- phase B part 2: batched-head attention (st PSUM freed) ----
        with tc.tile_pool(name="sbB2", bufs=1) as sbB2, \
             tc.tile_pool(name="psB2", bufs=1, space="PSUM") as psB2:
            pstnT = psB2.tile([128, 512], f32, tag="pstnT")
            for p in range(4):
                nc.tensor.transpose(pstnT[:, p * 128:(p + 1) * 128],
                                    stn[:, p * 128:(p + 1) * 128], idf32[:])
            stnT = sbB2.tile([128, 512], f32)
            nc.vector.tensor_copy(stnT[:], pstnT[:])
            # stnT layout: partitions (j, dh), free (p, j', g); head h=2p+j
            # lives at the diagonal block [j*64:, p*128+j*64:]
            stnT_r = stnT[:].rearrange("p (a b) -> p a b", a=4)
            pqk = psB2.tile([128, 512], f32, tag="pqk")
            pv = psB2.tile([128, 256], f32, tag="pv")
            for j in range(2):
                rhs_j = stnT_r[j * 64:(j + 1) * 64, :, j * 64:(j + 1) * 64]
                nc.tensor.matmul(pqk[j * 64:(j + 1) * 64, 0:256],
                                 wqkv_sb[j * 64:(j + 1) * 64, 0:64],
                                 rhs_j, start=True, stop=True)
                nc.tensor.matmul(pqk[j * 64:(j + 1) * 64, 256:512],
                                 wqkv_sb[j * 64:(j + 1) * 64, 64:128],
                                 rhs_j, start=True, stop=True)
            qk_sb = sbB2.tile([128, 512], f32)
            nc.vector.tensor_copy(qk_sb[:], pqk[:])
            for h in range(8):
                p_, j = h // 2, h % 2
                nc.tensor.matmul(pv[j * 64:(j + 1) * 64, p_ * 64:(p_ + 1) * 64],
                                 stnT[j * 64:(j + 1) * 64,
                                      p_ * 128 + j * 64: p_ * 128 + j * 64 + 64],
                                 wqkv_sb[j * 64:(j + 1) * 64, 128:192],
                                 start=True, stop=True)
            v_sb = sbB2.tile([128, 256], f32)
            nc.vector.tensor_copy(v_sb[:], pv[:])
            # A.T[g',g] per head, packed [ (j,g'), (p,g) ]
            pa = psB2.tile([128, 256], f32, tag="pa")
            for h in range(8):
                p_, j = h // 2, h % 2
                nc.tensor.matmul(pa[j * 64:(j + 1) * 64, p_ * 64:(p_ + 1) * 64],
                                 qk_sb[j * 64:(j + 1) * 64, 256 + p_ * 64: 256 + (p_ + 1) * 64],
                                 qk_sb[j * 64:(j + 1) * 64, p_ * 64:(p_ + 1) * 64],
                                 start=True, stop=True)
            ea = sbB2.tile([128, 256], f32)
            nc.scalar.activation(ea[:], pa[:], AF.Exp, scale=0.125)
            ps = psB2.tile([128, 4], f32, tag="ps")
            po = psB2.tile([128, 256], f32, tag="po")
            for h in range(8):
                p_, j = h // 2, h % 2
                ea_h = ea[j * 64:(j + 1) * 64, p_ * 64:(p_ + 1) * 64]
                nc.tensor.matmul(ps[j * 64:(j + 1) * 64, p_: p_ + 1],
                                 ea_h, ones_sb[j * 64:(j + 1) * 64, 0:1],
                                 start=True, stop=True)
                nc.tensor.matmul(po[j * 64:(j + 1) * 64, p_ * 64:(p_ + 1) * 64],
                                 v_sb[j * 64:(j + 1) * 64, p_ * 64:(p_ + 1) * 64],
                                 ea_h, start=True, stop=True)
            rs = sbB2.tile([128, 4], f32)
            nc.vector.reciprocal(rs[:], ps[:])
            o_sb = sbB2.tile([128, 256], f32)
            nc.vector.tensor_copy(o_sb[:], po[:])
            pM = psB2.tile([128, 512], f32, tag="pM")
            for h in range(8):
                p_, j = h // 2, h % 2
                nc.tensor.matmul(pM[j * 64:(j + 1) * 64, p_ * 128:(p_ + 1) * 128],
                                 o_sb[j * 64:(j + 1) * 64, p_ * 64:(p_ + 1) * 64],
                                 owt_sb[j * 64:(j + 1) * 64, h * 128:(h + 1) * 128],
                                 start=True, stop=True)
            for p_ in range(4):
                nc.vector.tensor_scalar_mul(M_sb[:, p_ * 128:(p_ + 1) * 128],
                                            pM[:, p_ * 128:(p_ + 1) * 128],
                                            rs[:, p_: p_ + 1])

        # ---- phase C ----
        with tc.tile_pool(name="sbC", bufs=3) as sbC, \
             tc.tile_pool(name="psC", bufs=3, space="PSUM") as psC:
            for i in range(32):
                po = psC.tile([128, 512], f32)
                for p in range(4):
                    nc.tensor.matmul(
                        po[:], M_sb[:, p * 128:(p + 1) * 128],
                        Wchm[:, p * 16384 + i * 512: p * 16384 + (i + 1) * 512],
                        start=(p == 0), stop=(p == 3))
                ob = sbC.tile([128, 512], f32)
                nc.vector.tensor_copy(ob[:], po[:])
                nc.sync.dma_start(outT[:, i * 512:(i + 1) * 512], ob[:])

    nc.compile()
    return nc


def _prep(inputs):
    x = np.asarray(inputs["x"], dtype=np.float32)
    conv_fx_w = np.asarray(inputs["conv_fx_w"], dtype=np.float32)
    conv_fx_b = np.asarray(inputs["conv_fx_b"], dtype=np.float32)
    conv_x_w = np.asarray(inputs["conv_x_w"], dtype=np.float32)
    conv_x_b = np.asarray(inputs["conv_x_b"], dtype=np.float32)
    slice_w = np.asarray(inputs["slice_w"], dtype=np.float32)
    slice_b = np.asarray(inputs["slice_b"], dtype=np.float32)
    temperature = np.asarray(inputs["temperature"], dtype=np.float32)
    wq = np.asarray(inputs["wq"], dtype=np.float32)
    wk = np.asarray(inputs["wk"], dtype=np.float32)
    wv = np.asarray(inputs["wv"], dtype=np.float32)
    out_w = np.asarray(inputs["out_w"], dtype=np.float32)

    # fold slice_w into the conv_x weights: the conv then emits logits
    # (pre-temperature) directly, channel (h, g)
    Wf = np.einsum("abchd,gd->abchg",
                   conv_x_w.reshape(3, 3, DIM, HEADS, DH),
                   slice_w).reshape(3, 3, DIM, HEADS * G)
    wx_np = np.ascontiguousarray(
        Wf.transpose(2, 0, 1, 3).reshape(128, 9 * 512)).astype(ml_dtypes.bfloat16)
    wfx_np = np.ascontiguousarray(
        conv_fx_w.transpose(2, 0, 1, 3).reshape(128, 9 * 512)).astype(ml_dtypes.bfloat16)

    temp = np.clip(temperature.reshape(HEADS), 0.1, 5.0)
    actv_np = np.zeros((128, 8), np.float32)
    for q in range(4):
        for j in range(2):
            h = 2 * q + j
            bias_fold = slice_b + slice_w @ conv_x_b[h * 64:(h + 1) * 64]
            actv_np[j * 64:(j + 1) * 64, q] = 1.0 / temp[h]
            actv_np[j * 64:(j + 1) * 64, 4 + q] = bias_fold / temp[h]

    wqkv_half = np.concatenate([wq.T, wk.T, wv.T], axis=1).astype(np.float32)
    wqkv_np = np.vstack([wqkv_half, wqkv_half])
    owt_half = np.ascontiguousarray(
        out_w.T.reshape(8, 64, 128).transpose(1, 0, 2).reshape(64, 1024))
    owt_np = np.vstack([owt_half, owt_half])
    bfx_np = np.ascontiguousarray(conv_fx_b.reshape(1, 512))

    in_maps = []
    for b in range(B):
        xi = x[b].reshape(HM, WM, DIM)
        xp = np.zeros((128, 130, 130), ml_dtypes.bfloat16)
        xp[:, 1:129, 1:129] = xi.transpose(2, 0, 1).astype(ml_dtypes.bfloat16)
        in_maps.append({
            "xTp": xp, "wx": wx_np, "wfx": wfx_np,
            "actv": actv_np, "wqkv": wqkv_np, "owt": owt_np, "bfxp": bfx_np,
        })
    return in_maps


def kernel(**inputs):
    from concourse.bass_utils import run_bass_kernel_spmd

    if "nc" not in _CACHE:
        _CACHE["nc"] = _build()
    nc = _CACHE["nc"]

    in_maps = _prep(inputs)
    res = run_bass_kernel_spmd(nc, in_maps, core_ids=list(range(NCORES)))
    out_b = np.asarray(inputs["out_b"], dtype=np.float32)
    out = np.empty((B, N, DIM), np.float32)
    for b in range(B):
        out[b] = res.results[b]["outT"].T + out_b
    return out


# revision 10
# speedup vs baseline: 1.1844x; 1.0118x over previous
"""PhysicsAttention (structured mesh 2D) Trainium2 kernel.

Data-parallel over batch: each of the 8 NeuronCores processes one batch
element end-to-end (no collectives).

Per-core pipeline (one batch element, mesh 128x128, N=16384 pixels):
  phase A (per row-tile of 4 image rows = 512 px):
    logits   : 3x3/128->512 conv with slice_w folded into the conv
               weights (bf16), channel-major out -> exp() directly off
               PSUM with per-partition 1/temp scale + folded bias
    eT       : PE transposes to pixel-major, per 128-px chunk
    softmax  : row sums (DVE reduce) + reciprocal -> w = e * (1/s)
    conv_fx  : same conv (bf16), pixel-major out (x-window stationary)
    slice_tok: accumulate st[g,c] += wT.T @ fx  and norm[g] += wT.T @ 1
    w_chm    : PE transpose w back to channel-major for phase C
  phase B: normalize slice tokens, then head-batched q/k/v, 64-token
    attention (no max-sub: logits are ~1e-3), fold out_slice with out_w
    into M[g, d]
  phase C: outT[d, n] = sum_g M[g,:].T @ w_chm[g, n]  (K=512, 4 blocks)

Host side: pads/transposes x (bf16), folds slice_w into conv_x weights,
folds conv_x bias + slice bias + temperature into the exp() scale/bias
vectors, adds out_b.
"""

import numpy as np
import ml_dtypes
from contextlib import ExitStack

B = 8
HM = WM = 128
DIM = 128
HEADS = 8
DH = 64
G = 64
INNER = 512
N = HM * WM
NCORES = 8
RT = 32  # row tiles (4 image rows each)

_CACHE = {}


def _build():
    import concourse.bass as bass
    import concourse.tile as tile
    from concourse import bacc, mybir
    from concourse.masks import make_identity

    f32 = mybir.dt.float32
    f32r = mybir.dt.float32r
    bf16 = mybir.dt.bfloat16
    AF = mybir.ActivationFunctionType
    AX = mybir.AxisListType

    ALU = mybir.AluOpType

    nc = bacc.Bacc("TRN2", target_bir_lowering=False, debug=False)
    xTp = nc.dram_tensor("xTp", [128, 130, 130], bf16, kind="ExternalInput").ap()
    wx = nc.dram_tensor("wx", [128, 9 * 512], bf16, kind="ExternalInput").ap()
    wfx = nc.dram_tensor("wfx", [128, 9 * 512], bf16, kind="ExternalInput").ap()
    actv = nc.dram_tensor("actv", [128, 8], f32, kind="ExternalInput").ap()
    wqkv = nc.dram_tensor("wqkv", [128, 192], f32, kind="ExternalInput").ap()
    owt = nc.dram_tensor("owt", [128, 1024], f32, kind="ExternalInput").ap()
    bfxp = nc.dram_tensor("bfxp", [128, 512], f32, kind="ExternalInput").ap()
    outT = nc.dram_tensor("outT", [128, 16384], f32, kind="ExternalOutput").ap()

    with tile.TileContext(nc) as tc, ExitStack() as top:
        consts = top.enter_context(tc.tile_pool(name="consts", bufs=1))
        wchmP = top.enter_context(tc.tile_pool(name="wchmP", bufs=1))

        # weight loads split into chunks so they spread over DMA queues
        wx_sb = consts.tile([128, 9 * 512], bf16)
        for c in range(4):
            nc.sync.dma_start(wx_sb[:, c * 1152:(c + 1) * 1152],
                              wx[:, c * 1152:(c + 1) * 1152])
        wfx_sb = consts.tile([128, 9 * 512], bf16)
        for c in range(4):
            nc.sync.dma_start(wfx_sb[:, c * 1152:(c + 1) * 1152],
                              wfx[:, c * 1152:(c + 1) * 1152])
        actv_sb = consts.tile([128, 8], f32)
        nc.sync.dma_start(actv_sb[:], actv[:])
        wqkv_sb = consts.tile([128, 192], f32)
        nc.sync.dma_start(wqkv_sb[:], wqkv[:])
        owt_sb = consts.tile([128, 1024], f32)
        nc.sync.dma_start(owt_sb[:], owt[:])
        bfx_sb = consts.tile([128, 512], f32)
        nc.sync.dma_start(bfx_sb[:], bfxp[:])
        idbf = consts.tile([128, 128], bf16)
        make_identity(nc, idbf[:])
        idf32 = consts.tile([128, 128], f32)
        make_identity(nc, idf32[:])
        ones_sb = consts.tile([128, 1], f32)
        nc.vector.memset(ones_sb[:], 1.0)
        M_sb = consts.tile([128, 512], bf16)
        stn = consts.tile([128, 512], f32)

        Wchm = wchmP.tile([128, 4 * 16384], bf16)

        with tc.tile_pool(name="stP", bufs=1, space="PSUM") as stP:
            # two banks, each holding two (g,c)-pair regions of width 129:
            # cols [0:128) = slice_token pair, col 128 = norm (ones-column)
            psum_st0 = stP.tile([128, 258], f32, tag="st0")
            psum_st1 = stP.tile([128, 258], f32, tag="st1")
            st_banks = (psum_st0, psum_st1)

            with tc.tile_pool(name="xwin", bufs=2) as xwinP, \
                 tc.tile_pool(name="sbA", bufs=3) as sbA, \
                 tc.tile_pool(name="psA", bufs=3, space="PSUM") as psA:
                for t in range(RT):
                    w6 = xwinP.tile([128, 6, 130], bf16)
                    nc.sync.dma_start(w6[:], xTp[:, 4 * t: 4 * t + 6, :])
                    e_list = []
                    for q in range(4):
                        pxm = psA.tile([128, 512], f32, tag="pA")
                        for tap in range(9):
                            ky, kx = tap // 3, tap % 3
                            nc.tensor.matmul(
                                pxm[:],
                                wx_sb[:, q * 1152 + tap * 128: q * 1152 + (tap + 1) * 128],
                                w6[:, ky: ky + 4, kx: kx + 128],
                                start=(tap == 0), stop=(tap == 8))
                        e_q = sbA.tile([128, 512], bf16, tag="e", bufs=6)
                        nc.scalar.activation(e_q[:], pxm[:], AF.Exp,
                                             bias=actv_sb[:, 4 + q: 5 + q],
                                             scale=actv_sb[:, q: q + 1])
                        e_list.append(e_q)
                    for k in range(4):
                        gch = 4 * t + k
                        peT = psA.tile([128, 512], bf16, tag="pB")
                        for q in range(4):
                            nc.tensor.transpose(peT[:, q * 128:(q + 1) * 128],
                                                e_list[q][:, k * 128:(k + 1) * 128],
                                                idbf[:])
                        s_k = sbA.tile([128, 8], f32, tag="s", bufs=4)
                        nc.vector.reduce_sum(
                            s_k[:], peT[:].rearrange("p (h g) -> p h g", h=8),
                            axis=AX.X)
                        r_k = sbA.tile([128, 8], f32, tag="r", bufs=4)
                        nc.vector.reciprocal(r_k[:], s_k[:])
                        wT = sbA.tile([128, 512], bf16, tag="wT", bufs=3)
                        r_b = bass.AP(tensor=r_k[:].tensor, offset=r_k[:].offset,
                                      ap=[r_k[:].ap[0], [1, 8], [0, 64]])
                        nc.vector.tensor_mul(wT[:], peT[:], r_b)
                        pfx = psA.tile([128, 512], f32, tag="pA")
                        for tap in range(9):
                            ky, kx = tap // 3, tap % 3
                            nc.tensor.matmul(
                                pfx[:],
                                w6[:, k + ky, kx: kx + 128],
                                wfx_sb[:, tap * 512:(tap + 1) * 512],
                                start=(tap == 0), stop=(tap == 8))
                        fx = sbA.tile([128, 4, 129], bf16, tag="fx", bufs=3)
                        nc.scalar.activation(
                            fx[:, :, 0:128],
                            pfx[:].rearrange("p (q n) -> p q n", q=4), AF.Copy)
                        nc.vector.memset(fx[:, :, 128:129], 1.0)
                        for p in range(4):
                            nc.tensor.matmul(
                                st_banks[p // 2][:, (p % 2) * 129:(p % 2) * 129 + 129],
                                wT[:, p * 128:(p + 1) * 128],
                                fx[:, p, :],
                                start=(gch == 0 and p % 2 == 0),
                                stop=(gch == 127 and p % 2 == 1))
                        pwc = psA.tile([128, 512], bf16, tag="pB")
                        for q in range(4):
                            nc.tensor.transpose(pwc[:, q * 128:(q + 1) * 128],
                                                wT[:, q * 128:(q + 1) * 128],
                                                idbf[:])
                        nc.vector.tensor_copy(
                            Wchm[:].rearrange("p (q n) -> p q n", q=4)[:, :, gch * 128:(gch + 1) * 128],
                            pwc[:])

            # ---- phase B part 1: normalized slice tokens (needs st PSUM) ----
            # stn = st/(norm+eps) + bfx; the exact form has the bias scaled
            # by norm/(norm+eps) (difference ~ bfx*4e-8, negligible)
            with tc.tile_pool(name="sbB1", bufs=1) as sbB1:
                norm_c = sbB1.tile([128, 4], f32)
                for b_ in range(2):
                    src = st_banks[b_][:]
                    nc.vector.tensor_copy(
                        norm_c[:, 2 * b_: 2 * b_ + 2],
                        bass.AP(tensor=src.tensor, offset=src.offset + 128,
                                ap=[src.ap[0], [129, 2]]))
                ne = sbB1.tile([128, 4], f32)
                nc.vector.tensor_scalar_add(ne[:], norm_c[:], 1e-5)
                rn = sbB1.tile([128, 4], f32)
                nc.vector.reciprocal(rn[:], ne[:])
                for p in range(4):
                    nc.vector.scalar_tensor_tensor(
                        stn[:, p * 128:(p + 1) * 128],
                        st_banks[p // 2][:, (p % 2) * 129:(p % 2) * 129 + 128],
                        rn[:, p: p + 1],
                        bfx_sb[:, p * 128:(p + 1) * 128],
                        ALU.mult, ALU.add)

        # ---- phase B part 2: batched-head attention (st PSUM freed) ----
        with tc.tile_pool(name="sbB2", bufs=1) as sbB2, \
             tc.tile_pool(name="psB2", bufs=1, space="PSUM") as psB2:
            pstnT = psB2.tile([128, 512], f32, tag="pstnT")
            for p in range(4):
                nc.tensor.transpose(pstnT[:, p * 128:(p + 1) * 128],
                                    stn[:, p * 128:(p + 1) * 128], idf32[:])
            stnT = sbB2.tile([128, 512], f32)
            nc.vector.tensor_copy(stnT[:], pstnT[:])
            # stnT layout: partitions (j, dh), free (p, j', g); head h=2p+j
            # lives at the diagonal block [j*64:, p*128+j*64:]
            stnT_r = stnT[:].rearrange("p (a b) -> p a b", a=4)
            pqk = psB2.tile([128, 512], f32, tag="pqk")
            pv = psB2.tile([128, 256], f32, tag="pv")
            for j in range(2):
                rhs_j = stnT_r[j * 64:(j + 1) * 64, :, j * 64:(j + 1) * 64]
                nc.tensor.matmul(pqk[j * 64:(j + 1) * 64, 0:256],
                                 wqkv_sb[j * 64:(j + 1) * 64, 0:64],
                                 rhs_j, start=True, stop=True)
                nc.tensor.matmul(pqk[j * 64:(j + 1) * 64, 256:512],
                                 wqkv_sb[j * 64:(j + 1) * 64, 64:128],
                                 rhs_j, start=True, stop=True)
            qk_sb = sbB2.tile([128, 512], f32)
            nc.vector.tensor_copy(qk_sb[:], pqk[:])
            for h in range(8):
                p_, j = h // 2, h % 2
                nc.tensor.matmul(pv[j * 64:(j + 1) * 64, p_ * 64:(p_ + 1) * 64],
                                 stnT[j * 64:(j + 1) * 64,
                                      p_ * 128 + j * 64: p_ * 128 + j * 64 + 64],
                                 wqkv_sb[j * 64:(j + 1) * 64, 128:192],
                                 start=True, stop=True)
            v_sb = sbB2.tile([128, 256], f32)
            nc.vector.tensor_copy(v_sb[:], pv[:])
            # A.T[g',g] per head, packed [ (j,g'), (p,g) ]
            pa = psB2.tile([128, 256], f32, tag="pa")
            for h in range(8):
                p_, j = h // 2, h % 2
                nc.tensor.matmul(pa[j * 64:(j + 1) * 64, p_ * 64:(p_ + 1) * 64],
                                 qk_sb[j * 64:(j + 1) * 64, 256 + p_ * 64: 256 + (p_ + 1) * 64],
                                 qk_sb[j * 64:(j + 1) * 64, p_ * 64:(p_ + 1) * 64],
                                 start=True, stop=True)
            ea = sbB2.tile([128, 256], f32)
            nc.scalar.activation(ea[:], pa[:], AF.Exp, scale=0.125)
            ps = psB2.tile([128, 4], f32, tag="ps")
            po = psB2.tile([128, 256], f32, tag="po")
            for h in range(8):
                p_, j = h // 2, h % 2
                ea_h = ea[j * 64:(j + 1) * 64, p_ * 64:(p_ + 1) * 64]
                nc.tensor.matmul(ps[j * 64:(j + 1) * 64, p_: p_ + 1],
                                 ea_h, ones_sb[j * 64:(j + 1) * 64, 0:1],
                                 start=True, stop=True)
                nc.tensor.matmul(po[j * 64:(j + 1) * 64, p_ * 64:(p_ + 1) * 64],
                                 v_sb[j * 64:(j + 1) * 64, p_ * 64:(p_ + 1) * 64],
                                 ea_h, start=True, stop=True)
            rs = sbB2.tile([128, 4], f32)
            nc.vector.reciprocal(rs[:], ps[:])
            o_sb = sbB2.tile([128, 256], f32)
            nc.vector.tensor_copy(o_sb[:], po[:])
            pM = psB2.tile([128, 512], f32, tag="pM")
            for h in range(8):
                p_, j = h // 2, h % 2
                nc.tensor.matmul(pM[j * 64:(j + 1) * 64, p_ * 128:(p_ + 1) * 128],
                                 o_sb[j * 64:(j + 1) * 64, p_ * 64:(p_ + 1) * 64],
                                 owt_sb[j * 64:(j + 1) * 64, h * 128:(h + 1) * 128],
                                 start=True, stop=True)
            for p_ in range(4):
                nc.vector.tensor_scalar_mul(M_sb[:, p_ * 128:(p_ + 1) * 128],
                                            pM[:, p_ * 128:(p_ + 1) * 128],
                                            rs[:, p_: p_ + 1])

        # ---- phase C ----
        with tc.tile_pool(name="sbC", bufs=6) as sbC, \
             tc.tile_pool(name="psC", bufs=6, space="PSUM") as psC:
            for i in range(32):
                po = psC.tile([128, 512], f32)
                for p in range(4):
                    nc.tensor.matmul(
                        po[:], M_sb[:, p * 128:(p + 1) * 128],
                        Wchm[:, p * 16384 + i * 512: p * 16384 + (i + 1) * 512],
                        start=(p == 0), stop=(p == 3))
                ob = sbC.tile([128, 512], f32)
                if i % 2 == 0:
                    nc.vector.tensor_copy(ob[:], po[:])
                else:
                    nc.scalar.activation(ob[:], po[:], AF.Copy)
                nc.sync.dma_start(outT[:, i * 512:(i + 1) * 512], ob[:])

    nc.compile()
    return nc


def _prep(inputs):
    x = np.asarray(inputs["x"], dtype=np.float32)
    conv_fx_w = np.asarray(inputs["conv_fx_w"], dtype=np.float32)
    conv_fx_b = np.asarray(inputs["conv_fx_b"], dtype=np.float32)
    conv_x_w = np.asarray(inputs["conv_x_w"], dtype=np.float32)
    conv_x_b = np.asarray(inputs["conv_x_b"], dtype=np.float32)
    slice_w = np.asarray(inputs["slice_w"], dtype=np.float32)
    slice_b = np.asarray(inputs["slice_b"], dtype=np.float32)
    temperature = np.asarray(inputs["temperature"], dtype=np.float32)
    wq = np.asarray(inputs["wq"], dtype=np.float32)
    wk = np.asarray(inputs["wk"], dtype=np.float32)
    wv = np.asarray(inputs["wv"], dtype=np.float32)
    out_w = np.asarray(inputs["out_w"], dtype=np.float32)

    # fold slice_w into the conv_x weights: the conv then emits logits
    # (pre-temperature) directly, channel (h, g)
    Wf = np.einsum("abchd,gd->abchg",
                   conv_x_w.reshape(3, 3, DIM, HEADS, DH),
                   slice_w).reshape(3, 3, DIM, HEADS * G)
    # q-major layout [cin, q, tap, 128] so the first conv only needs chunk 0
    wx_np = np.ascontiguousarray(
        Wf.transpose(2, 0, 1, 3).reshape(128, 9, 4, 128).transpose(0, 2, 1, 3)
        .reshape(128, 9 * 512)).astype(ml_dtypes.bfloat16)
    wfx_np = np.ascontiguousarray(
        conv_fx_w.transpose(2, 0, 1, 3).reshape(128, 9 * 512)).astype(ml_dtypes.bfloat16)

    temp = np.clip(temperature.reshape(HEADS), 0.1, 5.0)
    actv_np = np.zeros((128, 8), np.float32)
    for q in range(4):
        for j in range(2):
            h = 2 * q + j
            bias_fold = slice_b + slice_w @ conv_x_b[h * 64:(h + 1) * 64]
            actv_np[j * 64:(j + 1) * 64, q] = 1.0 / temp[h]
            actv_np[j * 64:(j + 1) * 64, 4 + q] = bias_fold / temp[h]

    wqkv_half = np.concatenate([wq.T, wk.T, wv.T], axis=1).astype(np.float32)
    wqkv_np = np.vstack([wqkv_half, wqkv_half])
    owt_half = np.ascontiguousarray(
        out_w.T.reshape(8, 64, 128).transpose(1, 0, 2).reshape(64, 1024))
    owt_np = np.vstack([owt_half, owt_half])
    bfx_np = np.ascontiguousarray(
        np.tile(conv_fx_b.reshape(1, 512), (128, 1)).astype(np.float32))

    in_maps = []
    for b in range(B):
        xi = x[b].reshape(HM, WM, DIM)
        xp = np.zeros((128, 130, 130), ml_dtypes.bfloat16)
        xp[:, 1:129, 1:129] = xi.transpose(2, 0, 1).astype(ml_dtypes.bfloat16)
        in_maps.append({
            "xTp": xp, "wx": wx_np, "wfx": wfx_np,
            "actv": actv_np, "wqkv": wqkv_np, "owt": owt_np, "bfxp": bfx_np,
        })
    return in_maps


def kernel(**inputs):
    from concourse.bass_utils import run_bass_kernel_spmd

    if "nc" not in _CACHE:
        _CACHE["nc"] = _build()
    nc = _CACHE["nc"]

    in_maps = _prep(inputs)
    res = run_bass_kernel_spmd(nc, in_maps, core_ids=list(range(NCORES)))
    out_b = np.asarray(inputs["out_b"], dtype=np.float32)
    out = np.empty((B, N, DIM), np.float32)
    for b in range(B):
        out[b] = res.results[b]["outT"].T + out_b
    return out


# revision 14
# speedup vs baseline: 1.1949x; 1.0089x over previous
"""PhysicsAttention (structured mesh 2D) Trainium2 kernel.

Data-parallel over batch: each of the 8 NeuronCores processes one batch
element end-to-end (no collectives).

Per-core pipeline (one batch element, mesh 128x128, N=16384 pixels):
  phase A (per row-tile of 4 image rows = 512 px):
    logits   : 3x3/128->512 conv with slice_w folded into the conv
               weights (bf16), channel-major out -> exp() directly off
               PSUM with per-partition 1/temp scale + folded bias
    eT       : PE transposes to pixel-major, per 128-px chunk
    softmax  : row sums (DVE reduce) + reciprocal -> w = e * (1/s)
    conv_fx  : same conv (bf16), pixel-major out (x-window stationary)
    slice_tok: accumulate st[g,c] += wT.T @ fx  and norm[g] += wT.T @ 1
    w_chm    : PE transpose w back to channel-major for phase C
  phase B: normalize slice tokens, then head-batched q/k/v, 64-token
    attention (no max-sub: logits are ~1e-3), fold out_slice with out_w
    into M[g, d]
  phase C: outT[d, n] = sum_g M[g,:].T @ w_chm[g, n]  (K=512, 4 blocks)

Host side: pads/transposes x (bf16), folds slice_w into conv_x weights,
folds conv_x bias + slice bias + temperature into the exp() scale/bias
vectors, adds out_b.
"""

import numpy as np
import ml_dtypes
from contextlib import ExitStack

B = 8
HM = WM = 128
DIM = 128
HEADS = 8
DH = 64
G = 64
INNER = 512
N = HM * WM
NCORES = 8
RT = 32  # row tiles (4 image rows each)

_CACHE = {}


def _build():
    import concourse.bass as bass
    import concourse.tile as tile
    from concourse import bacc, mybir
    from concourse.masks import make_identity

    f32 = mybir.dt.float32
    f32r = mybir.dt.float32r
    bf16 = mybir.dt.bfloat16
    AF = mybir.ActivationFunctionType
    AX = mybir.AxisListType

    ALU = mybir.AluOpType

    nc = bacc.Bacc("TRN2", target_bir_lowering=False, debug=False)
    xTp = nc.dram_tensor("xTp", [128, 130, 130], bf16, kind="ExternalInput").ap()
    wx = nc.dram_tensor("wx", [128, 9 * 512], bf16, kind="ExternalInput").ap()
    wfx = nc.dram_tensor("wfx", [128, 9 * 512], bf16, kind="ExternalInput").ap()
    actv = nc.dram_tensor("actv", [128, 8], f32, kind="ExternalInput").ap()
    wqkv = nc.dram_tensor("wqkv", [128, 192], f32, kind="ExternalInput").ap()
    owt = nc.dram_tensor("owt", [128, 1024], f32, kind="ExternalInput").ap()
    bfxp = nc.dram_tensor("bfxp", [128, 512], f32, kind="ExternalInput").ap()
    outT = nc.dram_tensor("outT", [128, 16384], f32, kind="ExternalOutput").ap()

    with tile.TileContext(nc) as tc, ExitStack() as top:
        consts = top.enter_context(tc.tile_pool(name="consts", bufs=1))
        wchmP = top.enter_context(tc.tile_pool(name="wchmP", bufs=1))

        # per-q weight tiles: conv for chunk q only depends on its own DMA
        wx_q = []
        for c in range(4):
            wxq_c = consts.tile([128, 1152], bf16, tag=f"wxq{c}")
            nc.sync.dma_start(wxq_c[:], wx[:, c * 1152:(c + 1) * 1152])
            wx_q.append(wxq_c)
        wfx_sb = consts.tile([128, 9 * 512], bf16)
        for c in range(4):
            nc.sync.dma_start(wfx_sb[:, c * 1152:(c + 1) * 1152],
                              wfx[:, c * 1152:(c + 1) * 1152])
        actv_sb = consts.tile([128, 8], f32)
        nc.sync.dma_start(actv_sb[:], actv[:])
        wqkv_sb = consts.tile([128, 192], f32)
        nc.sync.dma_start(wqkv_sb[:], wqkv[:])
        owt_sb = consts.tile([128, 1024], f32)
        nc.sync.dma_start(owt_sb[:], owt[:])
        bfx_sb = consts.tile([128, 512], f32)
        nc.sync.dma_start(bfx_sb[:], bfxp[:])
        idbf = consts.tile([128, 128], bf16)
        make_identity(nc, idbf[:])
        idf32 = consts.tile([128, 128], f32)
        make_identity(nc, idf32[:])
        ones_sb = consts.tile([128, 1], f32)
        nc.vector.memset(ones_sb[:], 1.0)
        M_t = []
        for p in range(4):
            M_p = consts.tile([128, 128], bf16, tag=f"M{p}")
            M_t.append(M_p)
        stn = consts.tile([128, 512], f32)

        Wchm = wchmP.tile([128, 4 * 16384], bf16)

        with tc.tile_pool(name="stP", bufs=1, space="PSUM") as stP:
            # two banks, each holding two (g,c)-pair regions of width 129:
            # cols [0:128) = slice_token pair, col 128 = norm (ones-column)
            psum_st0 = stP.tile([128, 258], f32, tag="st0")
            psum_st1 = stP.tile([128, 258], f32, tag="st1")
            st_banks = (psum_st0, psum_st1)

            with tc.tile_pool(name="xwin", bufs=2) as xwinP, \
                 tc.tile_pool(name="sbA", bufs=3) as sbA, \
                 tc.tile_pool(name="psA", bufs=3, space="PSUM") as psA:
                for t in range(RT):
                    w6 = xwinP.tile([128, 6, 130], bf16)
                    nc.sync.dma_start(w6[:], xTp[:, 4 * t: 4 * t + 6, :])
                    e_list = []
                    for q in range(4):
                        pxm = psA.tile([128, 512], f32, tag="pA")
                        for tap in range(9):
                            ky, kx = tap // 3, tap % 3
                            nc.tensor.matmul(
                                pxm[:],
                                wx_q[q][:, tap * 128:(tap + 1) * 128],
                                w6[:, ky: ky + 4, kx: kx + 128],
                                start=(tap == 0), stop=(tap == 8))
                        e_q = sbA.tile([128, 512], bf16, tag="e", bufs=6)
                        nc.scalar.activation(e_q[:], pxm[:], AF.Exp,
                                             bias=actv_sb[:, 4 + q: 5 + q],
                                             scale=actv_sb[:, q: q + 1])
                        e_list.append(e_q)
                    for k in range(4):
                        gch = 4 * t + k
                        peT = psA.tile([128, 512], bf16, tag="pB")
                        for q in range(4):
                            nc.tensor.transpose(peT[:, q * 128:(q + 1) * 128],
                                                e_list[q][:, k * 128:(k + 1) * 128],
                                                idbf[:])
                        s_k = sbA.tile([128, 8], f32, tag="s", bufs=4)
                        nc.vector.reduce_sum(
                            s_k[:], peT[:].rearrange("p (h g) -> p h g", h=8),
                            axis=AX.X)
                        r_k = sbA.tile([128, 8], f32, tag="r", bufs=4)
                        nc.vector.reciprocal(r_k[:], s_k[:])
                        wT = sbA.tile([128, 512], bf16, tag="wT", bufs=3)
                        r_b = bass.AP(tensor=r_k[:].tensor, offset=r_k[:].offset,
                                      ap=[r_k[:].ap[0], [1, 8], [0, 64]])
                        nc.vector.tensor_mul(wT[:], peT[:], r_b)
                        pfx = psA.tile([128, 512], f32, tag="pA")
                        for tap in range(9):
                            ky, kx = tap // 3, tap % 3
                            nc.tensor.matmul(
                                pfx[:],
                                w6[:, k + ky, kx: kx + 128],
                                wfx_sb[:, tap * 512:(tap + 1) * 512],
                                start=(tap == 0), stop=(tap == 8))
                        fx = sbA.tile([128, 4, 129], bf16, tag="fx", bufs=3)
                        nc.scalar.activation(
                            fx[:, :, 0:128],
                            pfx[:].rearrange("p (q n) -> p q n", q=4), AF.Copy)
                        nc.vector.memset(fx[:, :, 128:129], 1.0)
                        for p in range(4):
                            nc.tensor.matmul(
                                st_banks[p // 2][:, (p % 2) * 129:(p % 2) * 129 + 129],
                                wT[:, p * 128:(p + 1) * 128],
                                fx[:, p, :],
                                start=(gch == 0 and p % 2 == 0),
                                stop=(gch == 127 and p % 2 == 1))
                        pwc = psA.tile([128, 512], bf16, tag="pB")
                        for q in range(4):
                            nc.tensor.transpose(pwc[:, q * 128:(q + 1) * 128],
                                                wT[:, q * 128:(q + 1) * 128],
                                                idbf[:])
                        nc.vector.tensor_copy(
                            Wchm[:].rearrange("p (q n) -> p q n", q=4)[:, :, gch * 128:(gch + 1) * 128],
                            pwc[:])

            # ---- phase B part 1: normalized slice tokens (needs st PSUM) ----
            # stn = st/(norm+eps) + bfx; the exact form has the bias scaled
            # by norm/(norm+eps) (difference ~ bfx*4e-8, negligible)
            with tc.tile_pool(name="sbB1", bufs=1) as sbB1:
                norm_c = sbB1.tile([128, 4], f32)
                for b_ in range(2):
                    src = st_banks[b_][:]
                    nc.vector.tensor_copy(
                        norm_c[:, 2 * b_: 2 * b_ + 2],
                        bass.AP(tensor=src.tensor, offset=src.offset + 128,
                                ap=[src.ap[0], [129, 2]]))
                ne = sbB1.tile([128, 4], f32)
                nc.vector.tensor_scalar_add(ne[:], norm_c[:], 1e-5)
                rn = sbB1.tile([128, 4], f32)
                nc.vector.reciprocal(rn[:], ne[:])
                for p in range(4):
                    nc.vector.scalar_tensor_tensor(
                        stn[:, p * 128:(p + 1) * 128],
                        st_banks[p // 2][:, (p % 2) * 129:(p % 2) * 129 + 128],
                        rn[:, p: p + 1],
                        bfx_sb[:, p * 128:(p + 1) * 128],
                        ALU.mult, ALU.add)

        # ---- phase B part 2: batched-head attention (st PSUM freed) ----
        with tc.tile_pool(name="sbB2", bufs=1) as sbB2, \
             tc.tile_pool(name="psB2", bufs=1, space="PSUM") as psB2:
            pstnT = psB2.tile([128, 512], f32, tag="pstnT")
            for p in range(4):
                nc.tensor.transpose(pstnT[:, p * 128:(p + 1) * 128],
                                    stn[:, p * 128:(p + 1) * 128], idf32[:])
            stnT = sbB2.tile([128, 512], f32)
            nc.vector.tensor_copy(stnT[:], pstnT[:])
            # stnT layout: partitions (j, dh), free (p, j', g); head h=2p+j
            # lives at the diagonal block [j*64:, p*128+j*64:]
            stnT_r = stnT[:].rearrange("p (a b) -> p a b", a=4)
            pqk = psB2.tile([128, 512], f32, tag="pqk")
            pv = psB2.tile([128, 256], f32, tag="pv")
            for j in range(2):
                rhs_j = stnT_r[j * 64:(j + 1) * 64, :, j * 64:(j + 1) * 64]
                nc.tensor.matmul(pqk[j * 64:(j + 1) * 64, 0:256],
                                 wqkv_sb[j * 64:(j + 1) * 64, 0:64],
                                 rhs_j, start=True, stop=True)
                nc.tensor.matmul(pqk[j * 64:(j + 1) * 64, 256:512],
                                 wqkv_sb[j * 64:(j + 1) * 64, 64:128],
                                 rhs_j, start=True, stop=True)
            qk_sb = sbB2.tile([128, 512], f32)
            nc.vector.tensor_copy(qk_sb[:], pqk[:])
            for h in range(8):
                p_, j = h // 2, h % 2
                nc.tensor.matmul(pv[j * 64:(j + 1) * 64, p_ * 64:(p_ + 1) * 64],
                                 stnT[j * 64:(j + 1) * 64,
                                      p_ * 128 + j * 64: p_ * 128 + j * 64 + 64],
                                 wqkv_sb[j * 64:(j + 1) * 64, 128:192],
                                 start=True, stop=True)
            v_sb = sbB2.tile([128, 256], f32)
            nc.vector.tensor_copy(v_sb[:], pv[:])
            # A.T[g',g] per head, packed [ (j,g'), (p,g) ]
            pa = psB2.tile([128, 256], f32, tag="pa")
            for h in range(8):
                p_, j = h // 2, h % 2
                nc.tensor.matmul(pa[j * 64:(j + 1) * 64, p_ * 64:(p_ + 1) * 64],
                                 qk_sb[j * 64:(j + 1) * 64, 256 + p_ * 64: 256 + (p_ + 1) * 64],
                                 qk_sb[j * 64:(j + 1) * 64, p_ * 64:(p_ + 1) * 64],
                                 start=True, stop=True)
            ea = sbB2.tile([128, 256], f32)
            nc.scalar.activation(ea[:], pa[:], AF.Exp, scale=0.125)
            ps = psB2.tile([128, 4], f32, tag="ps")
            po = psB2.tile([128, 256], f32, tag="po")
            for h in range(8):
                p_, j = h // 2, h % 2
                ea_h = ea[j * 64:(j + 1) * 64, p_ * 64:(p_ + 1) * 64]
                nc.tensor.matmul(ps[j * 64:(j + 1) * 64, p_: p_ + 1],
                                 ea_h, ones_sb[j * 64:(j + 1) * 64, 0:1],
                                 start=True, stop=True)
                nc.tensor.matmul(po[j * 64:(j + 1) * 64, p_ * 64:(p_ + 1) * 64],
                                 v_sb[j * 64:(j + 1) * 64, p_ * 64:(p_ + 1) * 64],
                                 ea_h, start=True, stop=True)
            rs = sbB2.tile([128, 4], f32)
            nc.vector.reciprocal(rs[:], ps[:])
            o_sb = sbB2.tile([128, 256], f32)
            nc.vector.tensor_copy(o_sb[:], po[:])
            pM = psB2.tile([128, 512], f32, tag="pM")
            for h in range(8):
                p_, j = h // 2, h % 2
                nc.tensor.matmul(pM[j * 64:(j + 1) * 64, p_ * 128:(p_ + 1) * 128],
                                 o_sb[j * 64:(j + 1) * 64, p_ * 64:(p_ + 1) * 64],
                                 owt_sb[j * 64:(j + 1) * 64, h * 128:(h + 1) * 128],
                                 start=True, stop=True)
            for p_ in range(4):
                nc.vector.tensor_scalar_mul(M_t[p_][:],
                                            pM[:, p_ * 128:(p_ + 1) * 128],
                                            rs[:, p_: p_ + 1])

        # ---- phase C ----
        with tc.tile_pool(name="sbC", bufs=6) as sbC, \
             tc.tile_pool(name="psC", bufs=6, space="PSUM") as psC:
            for i in range(32):
                po = psC.tile([128, 512], f32)
                for p in range(4):
                    nc.tensor.matmul(
                        po[:], M_t[p][:],
                        Wchm[:, p * 16384 + i * 512: p * 16384 + (i + 1) * 512],
                        start=(p == 0), stop=(p == 3))
                ob = sbC.tile([128, 512], f32)
                if i % 2 == 0:
                    nc.vector.tensor_copy(ob[:], po[:])
                else:
                    nc.scalar.activation(ob[:], po[:], AF.Copy)
                nc.sync.dma_start(outT[:, i * 512:(i + 1) * 512], ob[:])

    nc.compile()
    return nc


def _prep(inputs):
    x = np.asarray(inputs["x"], dtype=np.float32)
    conv_fx_w = np.asarray(inputs["conv_fx_w"], dtype=np.float32)
    conv_fx_b = np.asarray(inputs["conv_fx_b"], dtype=np.float32)
    conv_x_w = np.asarray(inputs["conv_x_w"], dtype=np.float32)
    conv_x_b = np.asarray(inputs["conv_x_b"], dtype=np.float32)
    slice_w = np.asarray(inputs["slice_w"], dtype=np.float32)
    slice_b = np.asarray(inputs["slice_b"], dtype=np.float32)
    temperature = np.asarray(inputs["temperature"], dtype=np.float32)
    wq = np.asarray(inputs["wq"], dtype=np.float32)
    wk = np.asarray(inputs["wk"], dtype=np.float32)
    wv = np.asarray(inputs["wv"], dtype=np.float32)
    out_w = np.asarray(inputs["out_w"], dtype=np.float32)

    # fold slice_w into the conv_x weights: the conv then emits logits
    # (pre-temperature) directly, channel (h, g)
    Wf = np.einsum("abchd,gd->abchg",
                   conv_x_w.reshape(3, 3, DIM, HEADS, DH),
                   slice_w).reshape(3, 3, DIM, HEADS * G)
    # q-major layout [cin, q, tap, 128] so the first conv only needs chunk 0
    wx_np = np.ascontiguousarray(
        Wf.transpose(2, 0, 1, 3).reshape(128, 9, 4, 128).transpose(0, 2, 1, 3)
        .reshape(128, 9 * 512)).astype(ml_dtypes.bfloat16)
    wfx_np = np.ascontiguousarray(
        conv_fx_w.transpose(2, 0, 1, 3).reshape(128, 9 * 512)).astype(ml_dtypes.bfloat16)

    temp = np.clip(temperature.reshape(HEADS), 0.1, 5.0)
    actv_np = np.zeros((128, 8), np.float32)
    for q in range(4):
        for j in range(2):
            h = 2 * q + j
            bias_fold = slice_b + slice_w @ conv_x_b[h * 64:(h + 1) * 64]
            actv_np[j * 64:(j + 1) * 64, q] = 1.0 / temp[h]
            actv_np[j * 64:(j + 1) * 64, 4 + q] = bias_fold / temp[h]

    wqkv_half = np.concatenate([wq.T, wk.T, wv.T], axis=1).astype(np.float32)
    wqkv_np = np.vstack([wqkv_half, wqkv_half])
    owt_half = np.ascontiguousarray(
        out_w.T.reshape(8, 64, 128).transpose(1, 0, 2).reshape(64, 1024))
    owt_np = np.vstack([owt_half, owt_half])
    bfx_np = np.ascontiguousarray(
        np.tile(conv_fx_b.reshape(1, 512), (128, 1)).astype(np.float32))

    in_maps = []
    for b in range(B):
        xi = x[b].reshape(HM, WM, DIM)
        xp = np.zeros((128, 130, 130), ml_dtypes.bfloat16)
        xp[:, 1:129, 1:129] = xi.transpose(2, 0, 1).astype(ml_dtypes.bfloat16)
        in_maps.append({
            "xTp": xp, "wx": wx_np, "wfx": wfx_np,
            "actv": actv_np, "wqkv": wqkv_np, "owt": owt_np, "bfxp": bfx_np,
        })
    return in_maps


def kernel(**inputs):
    from concourse.bass_utils import run_bass_kernel_spmd

    if "nc" not in _CACHE:
        _CACHE["nc"] = _build()
    nc = _CACHE["nc"]

    in_maps = _prep(inputs)
    res = run_bass_kernel_spmd(nc, in_maps, core_ids=list(range(NCORES)))
    out_b = np.asarray(inputs["out_b"], dtype=np.float32)
    out = np.empty((B, N, DIM), np.float32)
    for b in range(B):
        out[b] = res.results[b]["outT"].T + out_b
    return out


# revision 15
# speedup vs baseline: 1.2101x; 1.0127x over previous
"""PhysicsAttention (structured mesh 2D) Trainium2 kernel.

Data-parallel over batch: each of the 8 NeuronCores processes one batch
element end-to-end (no collectives).

Per-core pipeline (one batch element, mesh 128x128, N=16384 pixels):
  phase A (per row-tile of 4 image rows = 512 px):
    logits   : 3x3/128->512 conv with slice_w folded into the conv
               weights (bf16), channel-major out -> exp() directly off
               PSUM with per-partition 1/temp scale + folded bias
    eT       : PE transposes to pixel-major, per 128-px chunk
    softmax  : row sums (DVE reduce) + reciprocal -> w = e * (1/s)
    conv_fx  : same conv (bf16), pixel-major out (x-window stationary)
    slice_tok: accumulate st[g,c] += wT.T @ fx  and norm[g] += wT.T @ 1
    w_chm    : PE transpose w back to channel-major for phase C
  phase B: normalize slice tokens, then head-batched q/k/v, 64-token
    attention (no max-sub: logits are ~1e-3), fold out_slice with out_w
    into M[g, d]
  phase C: outT[d, n] = sum_g M[g,:].T @ w_chm[g, n]  (K=512, 4 blocks)

Host side: pads/transposes x (bf16), folds slice_w into conv_x weights,
folds conv_x bias + slice bias + temperature into the exp() scale/bias
vectors, adds out_b.
"""

import numpy as np
import ml_dtypes
from contextlib import ExitStack

B = 8
HM = WM = 128
DIM = 128
HEADS = 8
DH = 64
G = 64
INNER = 512
N = HM * WM
NCORES = 8
RT = 32  # row tiles (4 image rows each)

_CACHE = {}


def _build():
    import concourse.bass as bass
    import concourse.tile as tile
    from concourse import bacc, mybir
    from concourse.masks import make_identity

    f32 = mybir.dt.float32
    f32r = mybir.dt.float32r
    bf16 = mybir.dt.bfloat16
    AF = mybir.ActivationFunctionType
    AX = mybir.AxisListType

    ALU = mybir.AluOpType

    nc = bacc.Bacc("TRN2", target_bir_lowering=False, debug=False)
    xTp = nc.dram_tensor("xTp", [128, 130, 130], bf16, kind="ExternalInput").ap()
    wx = nc.dram_tensor("wx", [128, 9 * 512], bf16, kind="ExternalInput").ap()
    wfx = nc.dram_tensor("wfx", [128, 9 * 512], bf16, kind="ExternalInput").ap()
    actv = nc.dram_tensor("actv", [128, 8], f32, kind="ExternalInput").ap()
    wqkv = nc.dram_tensor("wqkv", [128, 192], f32, kind="ExternalInput").ap()
    owt = nc.dram_tensor("owt", [128, 1024], f32, kind="ExternalInput").ap()
    bfxp = nc.dram_tensor("bfxp", [128, 512], f32, kind="ExternalInput").ap()
    outT = nc.dram_tensor("outT", [128, 16384], f32, kind="ExternalOutput").ap()

    with tile.TileContext(nc) as tc, ExitStack() as top:
        consts = top.enter_context(tc.tile_pool(name="consts", bufs=1))
        wchmP = top.enter_context(tc.tile_pool(name="wchmP", bufs=1))
        xwinP = top.enter_context(tc.tile_pool(name="xwin", bufs=2))

        # DMA issue order == need order: the first conv (t=0, q=0) needs
        # only wx chunk 0 + the first x window + actv, so those go first
        wx_q = []
        wxq_0 = consts.tile([128, 1152], bf16, tag="wxq0")
        nc.sync.dma_start(wxq_0[:], wx[:, 0:1152])
        wx_q.append(wxq_0)
        w6_0 = xwinP.tile([128, 6, 130], bf16)
        nc.sync.dma_start(w6_0[:], xTp[:, 0:6, :])
        actv_sb = consts.tile([128, 8], f32)
        nc.sync.dma_start(actv_sb[:], actv[:])
        for c in range(1, 4):
            wxq_c = consts.tile([128, 1152], bf16, tag=f"wxq{c}")
            nc.sync.dma_start(wxq_c[:], wx[:, c * 1152:(c + 1) * 1152])
            wx_q.append(wxq_c)
        wfx_sb = consts.tile([128, 9 * 512], bf16)
        for c in range(4):
            nc.sync.dma_start(wfx_sb[:, c * 1152:(c + 1) * 1152],
                              wfx[:, c * 1152:(c + 1) * 1152])
        wqkv_sb = consts.tile([128, 192], f32)
        nc.sync.dma_start(wqkv_sb[:], wqkv[:])
        owt_sb = consts.tile([128, 1024], f32)
        nc.sync.dma_start(owt_sb[:], owt[:])
        bfx_sb = consts.tile([128, 512], f32)
        nc.sync.dma_start(bfx_sb[:], bfxp[:])
        idbf = consts.tile([128, 128], bf16)
        make_identity(nc, idbf[:])
        idf32 = consts.tile([128, 128], f32)
        make_identity(nc, idf32[:])
        ones_sb = consts.tile([128, 1], f32)
        nc.vector.memset(ones_sb[:], 1.0)
        M_t = []
        for p in range(4):
            M_p = consts.tile([128, 128], bf16, tag=f"M{p}")
            M_t.append(M_p)
        stn = consts.tile([128, 512], f32)

        Wchm = wchmP.tile([128, 4 * 16384], bf16)

        with tc.tile_pool(name="stP", bufs=1, space="PSUM") as stP:
            # two banks, each holding two (g,c)-pair regions of width 129:
            # cols [0:128) = slice_token pair, col 128 = norm (ones-column)
            psum_st0 = stP.tile([128, 258], f32, tag="st0")
            psum_st1 = stP.tile([128, 258], f32, tag="st1")
            st_banks = (psum_st0, psum_st1)

            with tc.tile_pool(name="sbA", bufs=3) as sbA, \
                 tc.tile_pool(name="psA", bufs=3, space="PSUM") as psA:
                for t in range(RT):
                    if t == 0:
                        w6 = w6_0
                    else:
                        w6 = xwinP.tile([128, 6, 130], bf16)
                        nc.sync.dma_start(w6[:], xTp[:, 4 * t: 4 * t + 6, :])
                    e_list = []
                    for q in range(4):
                        pxm = psA.tile([128, 512], f32, tag="pA")
                        for tap in range(9):
                            ky, kx = tap // 3, tap % 3
                            nc.tensor.matmul(
                                pxm[:],
                                wx_q[q][:, tap * 128:(tap + 1) * 128],
                                w6[:, ky: ky + 4, kx: kx + 128],
                                start=(tap == 0), stop=(tap == 8))
                        e_q = sbA.tile([128, 512], bf16, tag="e", bufs=6)
                        nc.scalar.activation(e_q[:], pxm[:], AF.Exp,
                                             bias=actv_sb[:, 4 + q: 5 + q],
                                             scale=actv_sb[:, q: q + 1])
                        e_list.append(e_q)
                    for k in range(4):
                        gch = 4 * t + k
                        peT = psA.tile([128, 512], bf16, tag="pB")
                        for q in range(4):
                            nc.tensor.transpose(peT[:, q * 128:(q + 1) * 128],
                                                e_list[q][:, k * 128:(k + 1) * 128],
                                                idbf[:])
                        s_k = sbA.tile([128, 8], f32, tag="s", bufs=4)
                        nc.vector.reduce_sum(
                            s_k[:], peT[:].rearrange("p (h g) -> p h g", h=8),
                            axis=AX.X)
                        r_k = sbA.tile([128, 8], f32, tag="r", bufs=4)
                        nc.vector.reciprocal(r_k[:], s_k[:])
                        wT = sbA.tile([128, 512], bf16, tag="wT", bufs=3)
                        r_b = bass.AP(tensor=r_k[:].tensor, offset=r_k[:].offset,
                                      ap=[r_k[:].ap[0], [1, 8], [0, 64]])
                        nc.vector.tensor_mul(wT[:], peT[:], r_b)
                        pfx = psA.tile([128, 512], f32, tag="pA")
                        for tap in range(9):
                            ky, kx = tap // 3, tap % 3
                            nc.tensor.matmul(
                                pfx[:],
                                w6[:, k + ky, kx: kx + 128],
                                wfx_sb[:, tap * 512:(tap + 1) * 512],
                                start=(tap == 0), stop=(tap == 8))
                        fx = sbA.tile([128, 4, 129], bf16, tag="fx", bufs=3)
                        nc.scalar.activation(
                            fx[:, :, 0:128],
                            pfx[:].rearrange("p (q n) -> p q n", q=4), AF.Copy)
                        nc.vector.memset(fx[:, :, 128:129], 1.0)
                        for p in range(4):
                            nc.tensor.matmul(
                                st_banks[p // 2][:, (p % 2) * 129:(p % 2) * 129 + 129],
                                wT[:, p * 128:(p + 1) * 128],
                                fx[:, p, :],
                                start=(gch == 0 and p % 2 == 0),
                                stop=(gch == 127 and p % 2 == 1))
                        pwc = psA.tile([128, 512], bf16, tag="pB")
                        for q in range(4):
                            nc.tensor.transpose(pwc[:, q * 128:(q + 1) * 128],
                                                wT[:, q * 128:(q + 1) * 128],
                                                idbf[:])
                        nc.vector.tensor_copy(
                            Wchm[:].rearrange("p (q n) -> p q n", q=4)[:, :, gch * 128:(gch + 1) * 128],
                            pwc[:])

            # ---- phase B part 1: normalized slice tokens (needs st PSUM) ----
            # stn = st/(norm+eps) + bfx; the exact form has the bias scaled
            # by norm/(norm+eps) (difference ~ bfx*4e-8, negligible)
            with tc.tile_pool(name="sbB1", bufs=1) as sbB1:
                norm_c = sbB1.tile([128, 4], f32)
                for b_ in range(2):
                    src = st_banks[b_][:]
                    nc.vector.tensor_copy(
                        norm_c[:, 2 * b_: 2 * b_ + 2],
                        bass.AP(tensor=src.tensor, offset=src.offset + 128,
                                ap=[src.ap[0], [129, 2]]))
                ne = sbB1.tile([128, 4], f32)
                nc.vector.tensor_scalar_add(ne[:], norm_c[:], 1e-5)
                rn = sbB1.tile([128, 4], f32)
                nc.vector.reciprocal(rn[:], ne[:])
                for p in range(4):
                    nc.vector.scalar_tensor_tensor(
                        stn[:, p * 128:(p + 1) * 128],
                        st_banks[p // 2][:, (p % 2) * 129:(p % 2) * 129 + 128],
                        rn[:, p: p + 1],
                        bfx_sb[:, p * 128:(p + 1) * 128],
                        ALU.mult, ALU.add)

        # ---- phase B part 2: batched-head attention (st PSUM freed) ----
        with tc.tile_pool(name="sbB2", bufs=1) as sbB2, \
             tc.tile_pool(name="psB2", bufs=1, space="PSUM") as psB2:
            pstnT = psB2.tile([128, 512], f32, tag="pstnT")
            for p in range(4):
                nc.tensor.transpose(pstnT[:, p * 128:(p + 1) * 128],
                                    stn[:, p * 128:(p + 1) * 128], idf32[:])
            stnT = sbB2.tile([128, 512], f32)
            nc.vector.tensor_copy(stnT[:], pstnT[:])
            # stnT layout: partitions (j, dh), free (p, j', g); head h=2p+j
            # lives at the diagonal block [j*64:, p*128+j*64:]
            stnT_r = stnT[:].rearrange("p (a b) -> p a b", a=4)
            pqk = psB2.tile([128, 512], f32, tag="pqk")
            pv = psB2.tile([128, 256], f32, tag="pv")
            for j in range(2):
                rhs_j = stnT_r[j * 64:(j + 1) * 64, :, j * 64:(j + 1) * 64]
                nc.tensor.matmul(pqk[j * 64:(j + 1) * 64, 0:256],
                                 wqkv_sb[j * 64:(j + 1) * 64, 0:64],
                                 rhs_j, start=True, stop=True)
                nc.tensor.matmul(pqk[j * 64:(j + 1) * 64, 256:512],
                                 wqkv_sb[j * 64:(j + 1) * 64, 64:128],
                                 rhs_j, start=True, stop=True)
            qk_sb = sbB2.tile([128, 512], f32)
            nc.vector.tensor_copy(qk_sb[:], pqk[:])
            for h in range(8):
                p_, j = h // 2, h % 2
                nc.tensor.matmul(pv[j * 64:(j + 1) * 64, p_ * 64:(p_ + 1) * 64],
                                 stnT[j * 64:(j + 1) * 64,
                                      p_ * 128 + j * 64: p_ * 128 + j * 64 + 64],
                                 wqkv_sb[j * 64:(j + 1) * 64, 128:192],
                                 start=True, stop=True)
            v_sb = sbB2.tile([128, 256], f32)
            nc.vector.tensor_copy(v_sb[:], pv[:])
            # A.T[g',g] per head, packed [ (j,g'), (p,g) ]
            pa = psB2.tile([128, 256], f32, tag="pa")
            for h in range(8):
                p_, j = h // 2, h % 2
                nc.tensor.matmul(pa[j * 64:(j + 1) * 64, p_ * 64:(p_ + 1) * 64],
                                 qk_sb[j * 64:(j + 1) * 64, 256 + p_ * 64: 256 + (p_ + 1) * 64],
                                 qk_sb[j * 64:(j + 1) * 64, p_ * 64:(p_ + 1) * 64],
                                 start=True, stop=True)
            ea = sbB2.tile([128, 256], f32)
            nc.scalar.activation(ea[:], pa[:], AF.Exp, scale=0.125)
            ps = psB2.tile([128, 4], f32, tag="ps")
            po = psB2.tile([128, 256], f32, tag="po")
            for h in range(8):
                p_, j = h // 2, h % 2
                ea_h = ea[j * 64:(j + 1) * 64, p_ * 64:(p_ + 1) * 64]
                nc.tensor.matmul(ps[j * 64:(j + 1) * 64, p_: p_ + 1],
                                 ea_h, ones_sb[j * 64:(j + 1) * 64, 0:1],
                                 start=True, stop=True)
                nc.tensor.matmul(po[j * 64:(j + 1) * 64, p_ * 64:(p_ + 1) * 64],
                                 v_sb[j * 64:(j + 1) * 64, p_ * 64:(p_ + 1) * 64],
                                 ea_h, start=True, stop=True)
            rs = sbB2.tile([128, 4], f32)
            nc.vector.reciprocal(rs[:], ps[:])
            o_sb = sbB2.tile([128, 256], f32)
            nc.vector.tensor_copy(o_sb[:], po[:])
            pM = psB2.tile([128, 512], f32, tag="pM")
            for h in range(8):
                p_, j = h // 2, h % 2
                nc.tensor.matmul(pM[j * 64:(j + 1) * 64, p_ * 128:(p_ + 1) * 128],
                                 o_sb[j * 64:(j + 1) * 64, p_ * 64:(p_ + 1) * 64],
                                 owt_sb[j * 64:(j + 1) * 64, h * 128:(h + 1) * 128],
                                 start=True, stop=True)
            for p_ in range(4):
                nc.vector.tensor_scalar_mul(M_t[p_][:],
                                            pM[:, p_ * 128:(p_ + 1) * 128],
                                            rs[:, p_: p_ + 1])

        # ---- phase C ----
        with tc.tile_pool(name="sbC", bufs=6) as sbC, \
             tc.tile_pool(name="psC", bufs=6, space="PSUM") as psC:
            for i in range(32):
                po = psC.tile([128, 512], f32)
                for p in range(4):
                    nc.tensor.matmul(
                        po[:], M_t[p][:],
                        Wchm[:, p * 16384 + i * 512: p * 16384 + (i + 1) * 512],
                        start=(p == 0), stop=(p == 3))
                ob = sbC.tile([128, 512], f32)
                if i % 2 == 0:
                    nc.vector.tensor_copy(ob[:], po[:])
                else:
                    nc.scalar.activation(ob[:], po[:], AF.Copy)
                nc.sync.dma_start(outT[:, i * 512:(i + 1) * 512], ob[:])

    nc.compile()
    return nc


def _prep(inputs):
    x = np.asarray(inputs["x"], dtype=np.float32)
    conv_fx_w = np.asarray(inputs["conv_fx_w"], dtype=np.float32)
    conv_fx_b = np.asarray(inputs["conv_fx_b"], dtype=np.float32)
    conv_x_w = np.asarray(inputs["conv_x_w"], dtype=np.float32)
    conv_x_b = np.asarray(inputs["conv_x_b"], dtype=np.float32)
    slice_w = np.asarray(inputs["slice_w"], dtype=np.float32)
    slice_b = np.asarray(inputs["slice_b"], dtype=np.float32)
    temperature = np.asarray(inputs["temperature"], dtype=np.float32)
    wq = np.asarray(inputs["wq"], dtype=np.float32)
    wk = np.asarray(inputs["wk"], dtype=np.float32)
    wv = np.asarray(inputs["wv"], dtype=np.float32)
    out_w = np.asarray(inputs["out_w"], dtype=np.float32)

    # fold slice_w into the conv_x weights: the conv then emits logits
    # (pre-temperature) directly, channel (h, g)
    Wf = np.einsum("abchd,gd->abchg",
                   conv_x_w.reshape(3, 3, DIM, HEADS, DH),
                   slice_w).reshape(3, 3, DIM, HEADS * G)
    # q-major layout [cin, q, tap, 128] so the first conv only needs chunk 0
    wx_np = np.ascontiguousarray(
        Wf.transpose(2, 0, 1, 3).reshape(128, 9, 4, 128).transpose(0, 2, 1, 3)
        .reshape(128, 9 * 512)).astype(ml_dtypes.bfloat16)
    wfx_np = np.ascontiguousarray(
        conv_fx_w.transpose(2, 0, 1, 3).reshape(128, 9 * 512)).astype(ml_dtypes.bfloat16)

    temp = np.clip(temperature.reshape(HEADS), 0.1, 5.0)
    actv_np = np.zeros((128, 8), np.float32)
    for q in range(4):
        for j in range(2):
            h = 2 * q + j
            bias_fold = slice_b + slice_w @ conv_x_b[h * 64:(h + 1) * 64]
            actv_np[j * 64:(j + 1) * 64, q] = 1.0 / temp[h]
            actv_np[j * 64:(j + 1) * 64, 4 + q] = bias_fold / temp[h]

    wqkv_half = np.concatenate([wq.T, wk.T, wv.T], axis=1).astype(np.float32)
    wqkv_np = np.vstack([wqkv_half, wqkv_half])
    owt_half = np.ascontiguousarray(
        out_w.T.reshape(8, 64, 128).transpose(1, 0, 2).reshape(64, 1024))
    owt_np = np.vstack([owt_half, owt_half])
    bfx_np = np.ascontiguousarray(
        np.tile(conv_fx_b.reshape(1, 512), (128, 1)).astype(np.float32))

    in_maps = []
    for b in range(B):
        xi = x[b].reshape(HM, WM, DIM)
        xp = np.zeros((128, 130, 130), ml_dtypes.bfloat16)
        xp[:, 1:129, 1:129] = xi.transpose(2, 0, 1).astype(ml_dtypes.bfloat16)
        in_maps.append({
            "xTp": xp, "wx": wx_np, "wfx": wfx_np,
            "actv": actv_np, "wqkv": wqkv_np, "owt": owt_np, "bfxp": bfx_np,
        })
    return in_maps


def kernel(**inputs):
    from concourse.bass_utils import run_bass_kernel_spmd

    if "nc" not in _CACHE:
        _CACHE["nc"] = _build()
    nc = _CACHE["nc"]

    in_maps = _prep(inputs)
    res = run_bass_kernel_spmd(nc, in_maps, core_ids=list(range(NCORES)))
    out_b = np.asarray(inputs["out_b"], dtype=np.float32)
    out = np.empty((B, N, DIM), np.float32)
    for b in range(B):
        out[b] = res.results[b]["outT"].T + out_b
    return out
